# revision 1
# baseline (speedup 1.0000x reference)
"""DispLoss kernel for Trainium2 (8 NeuronCores, Bass/Tile).

Math notes
----------
reference computes, per pixel p (B*H*W of them):
    target = w_idx - disp
    mask   = valid & (disp < 192)
    pos    = clip(target + 0.1*W, 0, 1.1*W) / (1.1*W/255)      in [0, 255)
    lb = floor(pos); hb = lb+1 (never clamped since pos < 255); wh = pos-lb
    logp   = log_softmax(logits[:, :, p], axis=channels)
    ce     = -( (1-wh)*logp[lb] + wh*logp[hb] )
    logits_loss = sum(ce*mask)/msum;  coord_loss = sum(|coord-target|*mask)/msum

Key identities used on device:
 *  logp[c] = x[c] - lse,  lse = log(sum_c exp(x_c))  (no max-subtraction
    needed: |x| <= ~7 for randn inputs, exp is safe in fp32)
 *  (1-wh)*x[lb] + wh*x[hb] = sum_c hat(pos-c) * x[c]
    with hat(d) = relu(1-|d|) = 1 - min(|d|, 1), so
    sum_c hat(pos-c)*x[c] = sum_c x[c] - sum_c min(|pos-c|,1)*x[c]
 *  masked-out pixels get pos := -10  =>  hat==0 for all c  => net 0.
Device therefore only produces 5 scalars per core:
    [ sum min(|pos-c|,1)*x,  sum x,  sum mask*lse,  sum mask, sum |coord-target|*mask ]
and the host combines them.

Layout: channels on partitions (2 halves of 128), pixels on the free axis.
Per-pixel sum_c exp goes through the tensor engine with the *pixels as
stationary weight columns* (stride-S access pattern) and a ones moving
vector, so each matmul deposits 128 pixels' sumexp into a psum *column* --
the (128, S*NK) psum tile ends up pixel-major with no restack pass.
"""

import os
import sys
from contextlib import ExitStack

import numpy as np

for _p in ("/opt/trn_rl_repo", "/root/.axon_site/_ro/trn_rl_repo"):
    if os.path.isdir(_p) and _p not in sys.path:
        sys.path.insert(0, _p)

B, H, W = 2, 384, 1216
NBINS = 256
NCORES = 8

# Device tiling configuration (full problem).
CFG = dict(B=B, NB=NBINS, HC=H // NCORES, W=W, CH=3072, S=24, WIN=1024)

X_BF16 = True          # cast logits fp32->bf16 during DMA (SWDGE)
ACT_A_NUM, ACT_A_DEN = 1, 1   # fraction of |pos-c| windows computed on ScalarE
SX_ENGINE = "vector"   # engine for the sum(x) copy-accumulate pass


def derived(cfg):
    PB = cfg["HC"] * cfg["W"]
    CH, S, WIN = cfg["CH"], cfg["S"], cfg["WIN"]
    NK = PB // CH
    NW = CH // WIN
    COLS = S * NK
    assert CH == 128 * S, (CH, S)
    assert NK * CH == PB, (NK, CH, PB)
    assert NW * WIN == CH, (NW, WIN, CH)
    return PB, NK, NW, COLS


def build_program(cfg, x_bf16=X_BF16, act_a=(ACT_A_NUM, ACT_A_DEN),
                  sx_engine=SX_ENGINE, parts=("pos", "apass", "stt", "sumexp", "sx")):
    import concourse.bacc as bacc
    import concourse.tile as tile
    from concourse import mybir

    AF = mybir.ActivationFunctionType
    OP = mybir.AluOpType
    f32 = mybir.dt.float32
    bf16 = mybir.dt.bfloat16
    xdt = bf16 if x_bf16 else f32

    Bc, NB = cfg["B"], cfg["NB"]
    PB, NK, NW, COLS = derived(cfg)
    CH, S, WIN = cfg["CH"], cfg["S"], cfg["WIN"]

    nc = bacc.Bacc("TRN2", target_bir_lowering=False)
    xl = nc.dram_tensor("xl", [Bc, NB, PB], f32, kind="ExternalInput")
    posm = nc.dram_tensor("posm", [Bc, NK, CH], f32, kind="ExternalInput")
    maskp = nc.dram_tensor("maskp", [128, Bc * COLS], f32, kind="ExternalInput")
    l1mp = nc.dram_tensor("l1mp", [128, Bc * COLS], f32, kind="ExternalInput")
    cneg = nc.dram_tensor("cneg", [2, 128, 1], f32, kind="ExternalInput")
    cpos = nc.dram_tensor("cpos", [2, 128, 1], f32, kind="ExternalInput")
    outp = nc.dram_tensor("outp", [1, 5], f32, kind="ExternalOutput")

    n_acc = Bc * NK * 2           # one accum column per stt instruction
    n_sx = Bc * NK * 2            # one accum column per sum-x instruction

    with ExitStack() as ctx:
        tc = ctx.enter_context(tile.TileContext(nc))
        consts = ctx.enter_context(tc.tile_pool(name="consts", bufs=1))
        xpool = ctx.enter_context(tc.tile_pool(name="xpool", bufs=3))
        epool = ctx.enter_context(tc.tile_pool(name="epool", bufs=3))
        apool = ctx.enter_context(tc.tile_pool(name="apool", bufs=4))
        ypool = ctx.enter_context(tc.tile_pool(name="ypool", bufs=2))
        pospool = ctx.enter_context(tc.tile_pool(name="pospool", bufs=2, space="PSUM"))
        accps = ctx.enter_context(tc.tile_pool(name="accps", bufs=1, space="PSUM"))
        smalls = ctx.enter_context(tc.tile_pool(name="smalls", bufs=1))

        ones_bf = consts.tile([128, 1], bf16)
        nc.vector.memset(ones_bf, 1.0)
        ones_f = consts.tile([128, 1], f32)
        nc.vector.memset(ones_f, 1.0)
        ones_row = consts.tile([1, 128], f32)
        nc.vector.memset(ones_row, 1.0)

        ccn, ccp = [], []
        for h in range(2):
            t1 = consts.tile([128, 1], f32, name=f"ccn{h}", tag=f"ccn{h}")
            nc.sync.dma_start(out=t1, in_=cneg[h])
            ccn.append(t1)
            t2 = consts.tile([128, 1], f32, name=f"ccp{h}", tag=f"ccp{h}")
            nc.sync.dma_start(out=t2, in_=cpos[h])
            ccp.append(t2)
        maskt = consts.tile([128, Bc * COLS], f32)
        nc.sync.dma_start(out=maskt, in_=maskp[:, :])
        l1t = consts.tile([128, Bc * COLS], f32)
        nc.sync.dma_start(out=l1t, in_=l1mp[:, :])

        sxrow = smalls.tile([1, n_sx], f32)
        lse_acc = accps.tile([128, Bc * COLS], f32)
        nc.vector.memset(lse_acc, 1.0)
        # Walrus rejects self-loading matmuls with >1 sync wait. These two
        # dummy matmuls make PE "observe" the DVE-memset constants up front
        # so no later matmul needs a DVE wait for them.
        dummy_ps = accps.tile([128, 1], f32)
        nc.tensor.matmul(out=dummy_ps, lhsT=ones_row, rhs=ones_row[0:1, 0:1],
                         start=True, stop=True)
        nc.tensor.matmul(out=dummy_ps[0:1, :], lhsT=ones_bf, rhs=ones_bf,
                         start=True, stop=True)
        accs = smalls.tile([128, n_acc], f32)
        sxa = smalls.tile([128, n_sx], f32)
        finals = smalls.tile([128, 5], f32)
        nc.vector.memset(finals, 0.0)

        widx = 0   # a-pass window counter (ACT/DVE split)
        ai = 0     # stt accumulator column index
        sxi = 0    # sum-x accumulator column index
        for b in range(Bc):
            for k in range(NK):
                # pos row for this chunk: single-partition staging tile so
                # the matmul moving operand sits at base partition 0
                pt = xpool.tile([1, CH], f32, tag="pt")
                nc.sync.dma_start(out=pt, in_=posm[b, k])
                xts, ets = [], []
                for h in range(2):
                    xt = xpool.tile([128, CH], xdt, tag="xt")
                    src = xl[b, 128 * h:128 * h + 128, CH * k:CH * (k + 1)]
                    if x_bf16:
                        nc.gpsimd.dma_start(out=xt, in_=src)
                    else:
                        nc.sync.dma_start(out=xt, in_=src)
                    xts.append(xt)
                    if "sumexp" in parts:
                        et = epool.tile([128, CH], bf16, tag="et")
                        nc.scalar.activation(out=et, in_=xt, func=AF.Exp)
                        ets.append(et)
                    # total sum of x
                    if "sx" not in parts:
                        pass
                    elif sx_engine == "gpsimd":
                        nc.gpsimd.tensor_reduce(
                            sxrow[0:1, sxi:sxi + 1], xt,
                            axis=mybir.AxisListType.XYZWC, op=OP.add)
                    else:
                        sxs = ypool.tile([128, CH], xdt, tag="sxs")
                        nc.vector.tensor_scalar(
                            sxs, xt, 1.0, None, OP.mult, OP.add,
                            accum_out=sxa[:, sxi:sxi + 1])
                    sxi += 1
                # per-pixel sumexp: pixels as stationary weight columns
                if "sumexp" not in parts:
                    ets = []
                er0 = ets[0].rearrange("p (m s) -> p s m", s=S) if ets else None
                er1 = ets[1].rearrange("p (m s) -> p s m", s=S) if ets else None
                for f in (range(S) if ets else ()):
                    col = b * COLS + k * S + f
                    nc.tensor.matmul(out=lse_acc[:, col:col + 1],
                                     lhsT=er0[:, f, :], rhs=ones_bf,
                                     start=True, stop=False)
                    nc.tensor.matmul(out=lse_acc[:, col:col + 1],
                                     lhsT=er1[:, f, :], rhs=ones_bf,
                                     start=False, stop=True)
                # |pos - c| and the min-weighted reduction, per window
                if "apass" not in parts:
                    continue
                ats = [apool.tile([128, CH], bf16, tag="at0", name="at0"),
                       apool.tile([128, CH], bf16, tag="at1", name="at1")]
                for wI in (range(NW) if "pos" in parts else ()):
                    w0 = wI * WIN
                    pos_ps = pospool.tile([128, WIN], f32, tag="pos")
                    # tiny pre-writer matmul absorbs the psum-slot WAR wait
                    # so the real broadcast matmuls carry only the DMA wait
                    nc.tensor.matmul(
                        out=pos_ps[:, 0:1], lhsT=ones_row,
                        rhs=ones_row[0:1, 0:1], start=True, stop=True)
                    # psum bank limit: each matmul write must stay in one
                    # 2KB bank -> split the broadcast at 512-fp32 boundaries
                    for q0 in range(0, WIN, 512):
                        q1 = min(q0 + 512, WIN)
                        nc.tensor.matmul(
                            out=pos_ps[:, q0:q1], lhsT=ones_row,
                            rhs=pt[0:1, w0 + q0:w0 + q1],
                            start=True, stop=True)
                    # one consumer engine per window so the pre-writer's WAR
                    # wait is a single semaphore
                    use_act = (widx * act_a[0]) % act_a[1] < act_a[0]
                    widx += 1
                    for h in range(2):
                        if use_act:
                            nc.scalar.activation(out=ats[h][:, w0:w0 + WIN],
                                                 in_=pos_ps,
                                                 func=AF.Abs, bias=ccn[h],
                                                 scale=1.0)
                        else:
                            nc.vector.tensor_scalar(
                                ats[h][:, w0:w0 + WIN], pos_ps, ccp[h], 0.0,
                                OP.subtract, OP.abs_max)
                if "stt" in parts:
                    for h in range(2):
                        yt = ypool.tile([128, CH], bf16, tag="yt")
                        nc.vector.scalar_tensor_tensor(
                            out=yt, in0=ats[h], scalar=1.0,
                            in1=xts[h],
                            op0=OP.min, op1=OP.mult,
                            accum_out=accs[:, ai:ai + 1])
                        ai += 1

        # epilogue: lse, masked sums, final partition reduction
        lse_sb = smalls.tile([128, Bc * COLS], f32)
        nc.scalar.activation(out=lse_sb, in_=lse_acc, func=AF.Ln)
        scr = smalls.tile([128, Bc * COLS], f32)
        nc.vector.scalar_tensor_tensor(
            out=scr, in0=lse_sb, scalar=1.0, in1=maskt,
            op0=OP.mult, op1=OP.mult, accum_out=finals[:, 2:3])
        scr2 = smalls.tile([128, Bc * COLS], f32)
        nc.vector.tensor_scalar(scr2, maskt, 1.0, None, OP.mult, OP.add,
                                accum_out=finals[:, 3:4])
        scr3 = smalls.tile([128, Bc * COLS], f32)
        nc.vector.tensor_scalar(scr3, l1t, 1.0, None, OP.mult, OP.add,
                                accum_out=finals[:, 4:5])
        if "stt" in parts:
            nc.vector.tensor_reduce(finals[:, 0:1], accs,
                                    axis=mybir.AxisListType.X, op=OP.add)
        sx_tot = None
        if "sx" in parts:
            if sx_engine == "gpsimd":
                sx_tot = smalls.tile([1, 1], f32)
                nc.vector.tensor_reduce(sx_tot, sxrow,
                                        axis=mybir.AxisListType.X, op=OP.add)
            else:
                nc.vector.tensor_reduce(finals[:, 1:2], sxa,
                                        axis=mybir.AxisListType.X, op=OP.add)
        fin_ps = accps.tile([1, 5], f32)
        nc.tensor.matmul(out=fin_ps, lhsT=ones_f, rhs=finals[:, 0:5],
                         start=True, stop=True)
        out_sb = smalls.tile([1, 5], f32)
        nc.scalar.activation(out=out_sb, in_=fin_ps, func=AF.Copy)
        if sx_tot is not None:
            nc.vector.tensor_copy(out_sb[0:1, 1:2], sx_tot)
        nc.sync.dma_start(out=outp[:, :], in_=out_sb)

    nc.compile()
    return nc


def host_prep(cfg, coord, coord_logits, disp, valid, n_cores):
    """Slice + preprocess inputs per core. Returns in_maps list."""
    Bc, NB, HC, Wc = cfg["B"], cfg["NB"], cfg["HC"], cfg["W"]
    PB, NK, NW, COLS = derived(cfg)
    CH, S = cfg["CH"], cfg["S"]

    coord = np.asarray(coord, np.float32)
    coord_logits = np.ascontiguousarray(np.asarray(coord_logits, np.float32))
    disp = np.asarray(disp, np.float32)
    valid = np.asarray(valid, bool)

    wcol = np.arange(Wc, dtype=np.float32)
    target = (wcol[None, None, :] - disp).astype(np.float32)
    mask = (valid & (disp < np.float32(192.0))).astype(np.float32)
    labels = np.clip(target + np.float32(0.1 * Wc), np.float32(0.0),
                     np.float32(1.1 * Wc)).astype(np.float32)
    interval = np.float32(1.1 * Wc / 255.0)
    pos = (labels / interval).astype(np.float32)
    posm = np.where(mask > 0, pos, np.float32(-10.0)).astype(np.float32)
    l1m = (np.abs(coord - target) * mask).astype(np.float32)

    # permutation (pixel index within one batch-slice -> (partition, col))
    idx = np.arange(PB)
    part = (idx % CH) // S
    colb = (idx // CH) * S + idx % S

    cvals = np.arange(256, dtype=np.float32).reshape(2, 128, 1)
    cneg = -cvals
    cpos = cvals

    in_maps = []
    for c in range(n_cores):
        r0, r1 = c * HC, (c + 1) * HC
        xl_c = np.ascontiguousarray(
            coord_logits[:, :, r0:r1, :]).reshape(Bc, NB, PB)
        posm_c = np.ascontiguousarray(
            posm[:, r0:r1, :]).reshape(Bc, NK, CH)
        maskp = np.zeros((128, Bc * COLS), np.float32)
        l1mp = np.zeros((128, Bc * COLS), np.float32)
        for b in range(Bc):
            maskp[part, b * COLS + colb] = mask[b, r0:r1, :].ravel()
            l1mp[part, b * COLS + colb] = l1m[b, r0:r1, :].ravel()
        in_maps.append(dict(xl=xl_c, posm=posm_c, maskp=maskp, l1mp=l1mp,
                            cneg=cneg, cpos=cpos))
    return in_maps


def combine(partials):
    """partials: list of (5,1) arrays per core -> (objective, coord, logits)."""
    tot = np.sum([p.reshape(5) for p in partials], axis=0, dtype=np.float64)
    minx, sx, masklse, msum, l1 = tot
    msum = msum + 1e-6
    coord_loss = l1 / msum
    interp = sx - minx           # sum of hat-weighted logits
    logits_loss = (masklse - interp) / msum
    objective = 0.1 * coord_loss + logits_loss
    return (np.float32(objective), np.float32(coord_loss),
            np.float32(logits_loss))


_prog_cache = {}


def _get_program(key=None):
    k = key or (X_BF16, ACT_A_NUM, ACT_A_DEN, SX_ENGINE)
    if k not in _prog_cache:
        _prog_cache[k] = build_program(CFG, x_bf16=k[0], act_a=(k[1], k[2]),
                                       sx_engine=k[3])
    return _prog_cache[k]


def kernel(coord, coord_logits, disp, valid):
    from concourse.bass_utils import run_bass_kernel_spmd

    nc = _get_program()
    in_maps = host_prep(CFG, coord, coord_logits, disp, valid, NCORES)
    res = run_bass_kernel_spmd(nc, in_maps, core_ids=list(range(NCORES)))
    partials = [r["outp"] for r in res.results]
    return combine(partials)


# ---------------------------------------------------------------------------
# numpy model of the device program (for validation in test harnesses)
def model_partials(cfg, in_map):
    """Emulate one core's device math in numpy (fp32-ish)."""
    Bc, NB = cfg["B"], cfg["NB"]
    PB, NK, NW, COLS = derived(cfg)
    xl = in_map["xl"].astype(np.float32)        # (B, NB, PB)
    posm = in_map["posm"].reshape(Bc, PB)
    minx = 0.0
    sx = float(xl.sum(dtype=np.float64))
    lse_cols = np.zeros((Bc, PB), np.float64)
    for b in range(Bc):
        d = np.abs(posm[b][None, :] - np.arange(NB, dtype=np.float32)[:, None])
        minx += float((np.minimum(d, 1.0) * xl[b]).sum(dtype=np.float64))
        lse_cols[b] = np.log(np.exp(xl[b]).sum(axis=0, dtype=np.float64))
    # mask*lse with the permuted mask
    idx = np.arange(PB)
    part = (idx % cfg["CH"]) // cfg["S"]
    colb = (idx // cfg["CH"]) * cfg["S"] + idx % cfg["S"]
    masklse = 0.0
    for b in range(Bc):
        m = in_map["maskp"][part, b * COLS + colb]
        masklse += float((m * lse_cols[b]).sum())
    msum = float(in_map["maskp"].sum(dtype=np.float64))
    l1 = float(in_map["l1mp"].sum(dtype=np.float64))
    return np.array([minx, sx, masklse, msum, l1], np.float64).reshape(5, 1)



# revision 11
# speedup vs baseline: 1.9615x; 1.9615x over previous
"""DispLoss kernel for Trainium2 (8 NeuronCores, Bass/Tile).

Math notes
----------
reference computes, per pixel p (B*H*W of them):
    target = w_idx - disp
    mask   = valid & (disp < 192)
    pos    = clip(target + 0.1*W, 0, 1.1*W) / (1.1*W/255)      in [0, 255)
    lb = floor(pos); hb = lb+1 (never clamped since pos < 255); wh = pos-lb
    logp   = log_softmax(logits[:, :, p], axis=channels)
    ce     = -( (1-wh)*logp[lb] + wh*logp[hb] )
    logits_loss = sum(ce*mask)/msum;  coord_loss = sum(|coord-target|*mask)/msum

Key identities used on device:
 *  logp[c] = x[c] - lse,  lse = log(sum_c exp(x_c))  (no max-subtraction
    needed: |x| <= ~7 for randn inputs, exp is safe in fp32)
 *  (1-wh)*x[lb] + wh*x[hb] = sum_c hat(pos-c) * x[c]
    with hat(d) = relu(1-|d|) = 1 - min(|d|, 1), so
    sum_c hat(pos-c)*x[c] = sum_c x[c] - sum_c min(|pos-c|,1)*x[c]
 *  masked-out pixels get pos := -10  =>  hat==0 for all c  => net 0.
Device produces 2 scalars per core:
    [ sum min(|pos-c|,1)*x,  sum mask*lse ]
Host computes sum(x) (from the same bf16 values the device consumes, so
the hat identity cancels exactly), mask count and the |coord-target| L1
term, and combines everything.

Layout: channels on partitions (2 halves of 128), pixels on the free axis.
Per-pixel sum_c exp goes through the tensor engine with the *pixels as
stationary weight columns* (stride-S access pattern) and a ones moving
vector, so each matmul deposits 128 pixels' sumexp into a psum *column* --
the (128, S*NK) psum tile ends up pixel-major with no restack pass.

Perf structure (vs the first working version):
 *  logits are cast fp32->bf16 on the HOST, so HBM traffic halves and the
    loads are plain HWDGE DMAs (no SWDGE cast path hogging GpSimd).
 *  pos is broadcast to 128 partitions by ONE K=2 matmul per 512-column
    psum slice: stationary = ones[2,128] fp16, moving = [pos_hi; pos_lo]
    fp16 rows.  pos_hi=rint(pos) is exact in fp16, |pos_lo|<=0.5 has
    <=1.2e-4 error, and PE accumulates the pair in fp32.  fp32 moving
    operands (quarter-rate on PE) never appear.
 *  the |pos-c| window pass is split between ScalarE (Abs activation) and
    DVE (tensor_scalar abs_max) by ACT_A to balance the two engines.
"""

import os
import sys
from contextlib import ExitStack

import numpy as np
import ml_dtypes

for _p in ("/opt/trn_rl_repo", "/root/.axon_site/_ro/trn_rl_repo"):
    if os.path.isdir(_p) and _p not in sys.path:
        sys.path.insert(0, _p)

B, H, W = 2, 384, 1216
NBINS = 256
NCORES = 8

# Device tiling configuration (full problem).
CFG = dict(B=B, NB=NBINS, HC=H // NCORES, W=W, CH=3072, S=24, WIN=1536)

ACT_A_NUM, ACT_A_DEN = 1, 6   # fraction of |pos-c| windows computed on ScalarE


_minabs_op = None


def _get_minabs_op():
    """Fused DVE op: out = min(|in0 - s0|, 1) * in1, accum_out = sum(out).

    One DVE instruction replaces the |pos-c| materialization + the separate
    min/mult/accumulate pass.  Registered into concourse.dve_ops at runtime
    (plain module-level list/dicts), with the uops sha computed on the fly
    so the DveOp drift pin is self-consistent.
    """
    global _minabs_op
    if _minabs_op is not None:
        return _minabs_op
    import concourse.dve_ops as dops
    from concourse.dve_ops import DveOp
    from concourse.dve_spec import (
        C0, One, Spec, Src0, Src1, lower, maxx, minn, _has_src1,
    )
    from concourse.dve_spec import AluOp
    from concourse.dve_uop import DveOpSpec

    name = "DISP_MINABS_X_ANT"
    if name in dops._SUB_OPCODE_FOR_NAME:
        _minabs_op = next(o for o in dops.OPS if o.name == name)
        return _minabs_op

    spec = Spec(
        body=minn(maxx(Src0 - C0, C0 - Src0), One) * Src1,
        accum=AluOp.ADD,
        reference=lambda in0, in1, c0, c1, c2: (
            lambda o: (o, o.sum(axis=-1, keepdims=True)))(
            np.minimum(np.abs(in0 - c0), 1.0) * in1),
    )
    row = max(dops._SUB_OPCODE_FOR_NAME.values()) + 1
    assert row < 0x20, "custom-DVE row field overflow"
    shas = {}
    for ver in ("v3", "v4"):
        try:
            uops = lower(spec, ver=ver)
            shas[ver] = DveOpSpec(name=name, opcode=row, uops=uops,
                                  rd1_en=_has_src1(spec)).sha(ver)
        except Exception:
            pass
    op = DveOp(name, spec, subdim=False, uops_sha=shas)
    dops.OPS.append(op)
    dops.CUSTOM_DVE_SPECS[name] = spec
    dops._SUB_OPCODE_FOR_NAME[name] = row
    _minabs_op = op
    return op


def derived(cfg):
    PB = cfg["HC"] * cfg["W"]
    CH, S, WIN = cfg["CH"], cfg["S"], cfg["WIN"]
    NK = PB // CH
    NW = CH // WIN
    COLS = S * NK
    assert CH == 128 * S, (CH, S)
    assert NK * CH == PB, (NK, CH, PB)
    assert NW * WIN == CH, (NW, WIN, CH)
    return PB, NK, NW, COLS


def build_program(cfg, act_a=(ACT_A_NUM, ACT_A_DEN),
                  parts=("pos", "apass", "stt", "sumexp")):
    import concourse.bacc as bacc
    import concourse.tile as tile
    from concourse import mybir

    AF = mybir.ActivationFunctionType
    OP = mybir.AluOpType
    f32 = mybir.dt.float32
    bf16 = mybir.dt.bfloat16
    f16 = mybir.dt.float16

    Bc, NB = cfg["B"], cfg["NB"]
    PB, NK, NW, COLS = derived(cfg)
    CH, S, WIN = cfg["CH"], cfg["S"], cfg["WIN"]
    minabs = _get_minabs_op()

    nc = bacc.Bacc("TRN2", target_bir_lowering=False)
    xl = nc.dram_tensor("xl", [Bc, NB, PB], bf16, kind="ExternalInput")
    posm = nc.dram_tensor("posm", [Bc, NK, 2, CH], f16, kind="ExternalInput")
    maskp = nc.dram_tensor("maskp", [128, Bc * COLS], f32, kind="ExternalInput")
    cneg = nc.dram_tensor("cneg", [2, 128, 1], f32, kind="ExternalInput")
    cpos = nc.dram_tensor("cpos", [2, 128, 1], f32, kind="ExternalInput")
    outp = nc.dram_tensor("outp", [1, 5], f32, kind="ExternalOutput")

    n_acc = Bc * NK * NW * 2      # one accum column per (window, half)

    with ExitStack() as ctx:
        tc = ctx.enter_context(tile.TileContext(nc))
        consts = ctx.enter_context(tc.tile_pool(name="consts", bufs=1))
        xpool = ctx.enter_context(tc.tile_pool(name="xpool", bufs=3))
        epool = ctx.enter_context(tc.tile_pool(name="epool", bufs=3))
        apool = ctx.enter_context(tc.tile_pool(name="apool", bufs=4))
        ypool = ctx.enter_context(tc.tile_pool(name="ypool", bufs=2))
        pospool = ctx.enter_context(tc.tile_pool(name="pospool", bufs=2, space="PSUM"))
        accps = ctx.enter_context(tc.tile_pool(name="accps", bufs=1, space="PSUM"))
        smalls = ctx.enter_context(tc.tile_pool(name="smalls", bufs=1))

        ones_bf = consts.tile([128, 1], bf16)
        nc.vector.memset(ones_bf, 1.0)
        ones_f = consts.tile([128, 1], f32)
        nc.vector.memset(ones_f, 1.0)
        ones_row = consts.tile([1, 128], f32)
        nc.vector.memset(ones_row, 1.0)
        ones2 = consts.tile([2, 128], f16)
        nc.vector.memset(ones2, 1.0)

        ccn, ccp = [], []
        for h in range(2):
            t1 = consts.tile([128, 1], f32, name=f"ccn{h}", tag=f"ccn{h}")
            nc.sync.dma_start(out=t1, in_=cneg[h])
            ccn.append(t1)
            t2 = consts.tile([128, 1], f32, name=f"ccp{h}", tag=f"ccp{h}")
            nc.sync.dma_start(out=t2, in_=cpos[h])
            ccp.append(t2)
        maskt = consts.tile([128, Bc * COLS], f32)
        nc.sync.dma_start(out=maskt, in_=maskp[:, :])

        lse_acc = accps.tile([128, Bc * COLS], f32)
        nc.vector.memset(lse_acc, 1.0)
        # Walrus rejects self-loading matmuls with >1 sync wait. These
        # dummy matmuls make PE "observe" the DVE-memset constants up front
        # so no later matmul needs a DVE wait for them.  They scribble on
        # lse_acc[:, 0:1], which the first real sumexp matmul (start=True)
        # clears again -- no separate psum bank needed.
        nc.tensor.matmul(out=lse_acc[:, 0:1], lhsT=ones_row,
                         rhs=ones_row[0:1, 0:1], start=True, stop=True)
        nc.tensor.matmul(out=lse_acc[0:1, 0:1], lhsT=ones_bf, rhs=ones_bf,
                         start=True, stop=True)
        nc.tensor.matmul(out=lse_acc[:, 0:1], lhsT=ones2, rhs=ones2[:, 0:1],
                         start=True, stop=True)
        accs = smalls.tile([128, n_acc], f32)
        finals = smalls.tile([128, 5], f32)
        nc.vector.memset(finals, 0.0)

        widx = 0   # a-pass window counter (ACT/DVE split)
        ai = 0     # stt accumulator column index
        for b in range(Bc):
            for k in range(NK):
                # pos hi/lo rows for this chunk: 2-partition staging tile,
                # the K=2 matmul sums the rows back into fp32 pos.
                pt = xpool.tile([2, CH], f16, tag="pt")
                nc.sync.dma_start(out=pt, in_=posm[b, k])
                xts, ets = [], []
                for h in range(2):
                    xt = xpool.tile([128, CH], bf16, tag="xt")
                    src = xl[b, 128 * h:128 * h + 128, CH * k:CH * (k + 1)]
                    nc.sync.dma_start(out=xt, in_=src)
                    xts.append(xt)
                    if "sumexp" in parts:
                        et = epool.tile([128, CH], bf16, tag="et")
                        nc.scalar.activation(out=et, in_=xt, func=AF.Exp)
                        ets.append(et)
                # per-pixel sumexp: pixels as stationary weight columns
                er0 = ets[0].rearrange("p (m s) -> p s m", s=S) if ets else None
                er1 = ets[1].rearrange("p (m s) -> p s m", s=S) if ets else None
                for f in (range(S) if ets else ()):
                    col = b * COLS + k * S + f
                    nc.tensor.matmul(out=lse_acc[:, col:col + 1],
                                     lhsT=er0[:, f, :], rhs=ones_bf,
                                     start=True, stop=False)
                    nc.tensor.matmul(out=lse_acc[:, col:col + 1],
                                     lhsT=er1[:, f, :], rhs=ones_bf,
                                     start=False, stop=True)
                # min(|pos - c|, 1) * x accumulation, per window.
                # ACT windows: ScalarE Abs(pos-c) then a DVE stt.
                # DVE windows: one fused custom-DVE instruction.
                if "apass" not in parts:
                    continue
                for wI in (range(NW) if "pos" in parts else ()):
                    w0 = wI * WIN
                    pos_ps = pospool.tile([128, WIN], f32, tag="pos")
                    # tiny pre-writer matmul absorbs the psum-slot WAR wait
                    # so the real broadcast matmuls carry only the DMA wait
                    nc.tensor.matmul(
                        out=pos_ps[:, 0:1], lhsT=ones2,
                        rhs=ones2[:, 0:1], start=True, stop=True)
                    # psum bank limit: each matmul write must stay in one
                    # 2KB bank -> split the broadcast at 512-fp32 boundaries
                    for q0 in range(0, WIN, 512):
                        q1 = min(q0 + 512, WIN)
                        nc.tensor.matmul(
                            out=pos_ps[:, q0:q1], lhsT=ones2,
                            rhs=pt[:, w0 + q0:w0 + q1], start=True, stop=True)
                    # one consumer engine per window so the pre-writer's WAR
                    # wait is a single semaphore
                    use_act = (widx * act_a[0]) % act_a[1] < act_a[0]
                    widx += 1
                    for h in range(2):
                        if "stt" not in parts:
                            continue
                        xw = xts[h][:, w0:w0 + WIN]
                        if use_act:
                            at = apool.tile([128, WIN], bf16, tag="at")
                            nc.scalar.activation(out=at, in_=pos_ps,
                                                 func=AF.Abs, bias=ccn[h],
                                                 scale=1.0)
                            yt = ypool.tile([128, WIN], bf16, tag="yt")
                            nc.vector.scalar_tensor_tensor(
                                out=yt, in0=at, scalar=1.0, in1=xw,
                                op0=OP.min, op1=OP.mult,
                                accum_out=accs[:, ai:ai + 1])
                        else:
                            yt = ypool.tile([128, WIN], bf16, tag="yt")
                            nc.vector._custom_dve(
                                minabs, out=yt, in0=pos_ps, in1=xw,
                                s0=ccp[h], accum_out=accs[:, ai:ai + 1])
                        ai += 1

        # epilogue: lse, masked sums, final partition reduction
        lse_sb = smalls.tile([128, Bc * COLS], f32)
        nc.scalar.activation(out=lse_sb, in_=lse_acc, func=AF.Ln)
        scr = smalls.tile([128, Bc * COLS], f32)
        nc.vector.scalar_tensor_tensor(
            out=scr, in0=lse_sb, scalar=1.0, in1=maskt,
            op0=OP.mult, op1=OP.mult, accum_out=finals[:, 2:3])
        if "stt" in parts:
            nc.vector.tensor_reduce(finals[:, 0:1], accs,
                                    axis=mybir.AxisListType.X, op=OP.add)
        fin_ps = pospool.tile([1, 5], f32, tag="pos")
        nc.tensor.matmul(out=fin_ps, lhsT=ones_f, rhs=finals[:, 0:5],
                         start=True, stop=True)
        out_sb = smalls.tile([1, 5], f32)
        nc.scalar.activation(out=out_sb, in_=fin_ps, func=AF.Copy)
        nc.sync.dma_start(out=outp[:, :], in_=out_sb)

    nc.compile()
    return nc


def host_prep(cfg, coord, coord_logits, disp, valid, n_cores):
    """Slice + preprocess inputs per core.

    Returns (in_maps, host_terms) where host_terms = (sx, msum, l1):
    the full-tensor sums the device no longer computes.  sx is summed from
    the SAME bf16 values the device consumes so the hat identity
    (interp = sx - minx) cancels exactly.
    """
    Bc, NB, HC, Wc = cfg["B"], cfg["NB"], cfg["HC"], cfg["W"]
    PB, NK, NW, COLS = derived(cfg)
    CH, S = cfg["CH"], cfg["S"]

    coord = np.asarray(coord, np.float32)
    disp = np.asarray(disp, np.float32)
    valid = np.asarray(valid, bool)
    logits_bf = np.asarray(coord_logits).astype(ml_dtypes.bfloat16)
    logits_bf = np.ascontiguousarray(logits_bf)

    wcol = np.arange(Wc, dtype=np.float32)
    target = (wcol[None, None, :] - disp).astype(np.float32)
    mask = (valid & (disp < np.float32(192.0))).astype(np.float32)
    labels = np.clip(target + np.float32(0.1 * Wc), np.float32(0.0),
                     np.float32(1.1 * Wc)).astype(np.float32)
    interval = np.float32(1.1 * Wc / 255.0)
    pos = (labels / interval).astype(np.float32)
    posm = np.where(mask > 0, pos, np.float32(-10.0)).astype(np.float32)
    l1m = (np.abs(coord - target) * mask).astype(np.float32)

    pos_hi = np.rint(posm).astype(np.float32)
    pos_lo = posm - pos_hi
    posm2 = np.stack([pos_hi, pos_lo], axis=0).astype(np.float16)  # (2,B,H,W)

    # permutation (pixel index within one batch-slice -> (partition, col))
    idx = np.arange(PB)
    part = (idx % CH) // S
    colb = (idx // CH) * S + idx % S

    cvals = np.arange(256, dtype=np.float32).reshape(2, 128, 1)
    cneg = -cvals
    cpos = cvals

    sx = 0.0
    in_maps = []
    for c in range(n_cores):
        r0, r1 = c * HC, (c + 1) * HC
        xl_c = np.ascontiguousarray(
            logits_bf[:, :, r0:r1, :]).reshape(Bc, NB, PB)
        sx += float(np.asarray(xl_c, dtype=np.float32).sum(dtype=np.float64))
        # posm2 per core: (B, NK, 2, CH) fp16 rows [hi, lo]
        p2 = posm2[:, :, r0:r1, :].reshape(2, Bc, NK, CH)
        posm_c = np.ascontiguousarray(p2.transpose(1, 2, 0, 3))
        maskp = np.zeros((128, Bc * COLS), np.float32)
        for b in range(Bc):
            maskp[part, b * COLS + colb] = mask[b, r0:r1, :].ravel()
        in_maps.append(dict(xl=xl_c, posm=posm_c, maskp=maskp,
                            cneg=cneg, cpos=cpos))
    msum = float(mask.sum(dtype=np.float64))
    l1 = float(l1m.sum(dtype=np.float64))
    return in_maps, (sx, msum, l1)


def combine(partials, host_terms):
    """partials: list of (5,1)/(1,5) device arrays; host_terms=(sx,msum,l1)."""
    sx, msum, l1 = host_terms
    tot = np.sum([np.asarray(p, np.float64).reshape(5) for p in partials],
                 axis=0)
    minx, _, masklse, _, _ = tot
    msum = msum + 1e-6
    coord_loss = l1 / msum
    interp = sx - minx           # sum of hat-weighted logits
    logits_loss = (masklse - interp) / msum
    objective = 0.1 * coord_loss + logits_loss
    return (np.float32(objective), np.float32(coord_loss),
            np.float32(logits_loss))


_prog_cache = {}


def _get_program(key=None):
    k = key or (ACT_A_NUM, ACT_A_DEN)
    if k not in _prog_cache:
        _prog_cache[k] = build_program(CFG, act_a=k)
    return _prog_cache[k]


def kernel(coord, coord_logits, disp, valid):
    from concourse.bass_utils import run_bass_kernel_spmd

    nc = _get_program()
    in_maps, host_terms = host_prep(CFG, coord, coord_logits, disp, valid,
                                    NCORES)
    res = run_bass_kernel_spmd(nc, in_maps, core_ids=list(range(NCORES)))
    partials = [r["outp"] for r in res.results]
    return combine(partials, host_terms)


# ---------------------------------------------------------------------------
# numpy model of the device program (for validation in test harnesses)
def model_partials(cfg, in_map):
    """Emulate one core's device math in numpy (fp32-ish)."""
    Bc, NB = cfg["B"], cfg["NB"]
    PB, NK, NW, COLS = derived(cfg)
    xl = np.asarray(in_map["xl"], dtype=np.float32)     # (B, NB, PB)
    p2 = np.asarray(in_map["posm"], dtype=np.float32)   # (B, NK, 2, CH)
    posm = (p2[:, :, 0, :] + p2[:, :, 1, :]).reshape(Bc, PB)
    minx = 0.0
    lse_cols = np.zeros((Bc, PB), np.float64)
    for b in range(Bc):
        d = np.abs(posm[b][None, :] - np.arange(NB, dtype=np.float32)[:, None])
        minx += float((np.minimum(d, 1.0) * xl[b]).sum(dtype=np.float64))
        lse_cols[b] = np.log(np.exp(xl[b]).sum(axis=0, dtype=np.float64))
    # mask*lse with the permuted mask
    idx = np.arange(PB)
    part = (idx % cfg["CH"]) // cfg["S"]
    colb = (idx // cfg["CH"]) * cfg["S"] + idx % cfg["S"]
    masklse = 0.0
    for b in range(Bc):
        m = in_map["maskp"][part, b * COLS + colb]
        masklse += float((m * lse_cols[b]).sum())
    return np.array([minx, 0.0, masklse, 0.0, 0.0], np.float64).reshape(5, 1)
